# revision 33
# baseline (speedup 1.0000x reference)
"""Trainium2 Bass kernel for nn_DMHA_3255585210402 (retrieval_knn DMHA).

Key algebraic fact: TOPK == NVK == 4, so jax.lax.top_k over the size-4 v_keys
axis selects *all* entries; the gather+sum over (DVH, TOPK) reduces to a
constant vector c = 2 * v_embed[0:4].sum(0), and compute_value_states
collapses to  v = x * c.

So the module is causal MHA (B=2, H=16, T=2048, HD=128, D=2048) with
elementwise-scaled V.  Sharding: 8 cores = 2 batches x 4 head-groups.

Final design (~280us HW vs 334us fp32r baseline; rel err 5.1e-3):
  * all matmul operands bf16 (psum stays f32): halves DMA/SBUF, and bf16
    runs 1 cycle/row at ANY moving width (fp32r needs >=256), enabling
    fine-grained causal tiles (diagonal widths 512/384/256/128).
  * triangular mask via DVE tensor_mul with a [128,128] tile (gpsimd
    affine_select was on the exp->o-matmul critical path).
  * softmax denominators: off-diagonal quads summed on DVE then one
    ones-matmul per quad (deferred 2 o-units so the PE never waits on the
    DVE adds); diagonal chunks get per-chunk ones-matmuls at their width.
  * phase B is ONE flat software pipeline over every (j, h, i) score unit
    with a global 4-deep skew (scores pool = 5 psum banks, ps_o 2,
    ps_sum 1): the exp-hiding lookahead never resets at head/chunk
    boundaries.  Normalize (recip -> gpsimd partition_broadcast -> DVE
    scalar_tensor_tensor) is deferred one head and flushed at the next
    head's o-unit 2.
  * outproj chains for j-1 injected mid-head into B(j) (PE-heavy,
    scalar-free work balances the exp-bound stretches); psum->sbuf casts
    on DVE so exp never queues behind them; output staged bf16, flushed
    in 4/4/4/2/1/1-dk DMA groups.
  * DMA: Wq||Wk fused per-dk chunks JIT-issued interleaved with the first
    matmul emissions (sync-engine DMA issue costs ~650ns each, ~2us
    completion latency); everything else as single batched issues; the
    gpsimd library DMA (~11us) deferred past the startup window.
  * phase A: psum groups of 4 banks (q/k separate); final tci uses
    one-bank-per-head chains so banks drain incrementally into phase B.
"""

import math

import numpy as np
import ml_dtypes

import concourse.bass as bass
import concourse.mybir as mybir
import concourse.tile as tile
from concourse import bacc
from concourse.bass_utils import run_bass_kernel_spmd

B, T, D = 2, 2048, 2048
H, HD = 16, 128
G = 4              # head-groups (cores per batch)
GH = H // G        # heads per core
GF = GH * HD       # projected features per core (512)
NCORES = 8
P = 128            # partitions
TQ = 512           # tq chunk width (psum bank / fp32 moving max)
F32 = mybir.dt.float32
F32R = mybir.dt.float32r
BF16 = mybir.dt.bfloat16

DK = D // P        # 16 contraction chunks for projections
NTQ = T // TQ      # 4 query chunks
NTK = T // P       # 16 key chunks
SKEW = 4           # scores-ahead-of-o software pipeline depth

BF = ml_dtypes.bfloat16


def _body(tc, xT, xg, wqk, woT, cT, bqkT, ones, tri, out):
    nc = tc.nc
    rsqrt_hd = 1.0 / math.sqrt(HD)
    mult = mybir.AluOpType.mult

    with (
        tc.tile_pool(name="const", bufs=1) as const,
        tc.tile_pool(name="res1", bufs=1) as res1,
    ):
        from concourse import library_config
        qT_sb = res1.tile([P, GH, T], BF16)   # q, transposed per head
        kT_sb = res1.tile([P, GH, T], BF16)
        # phase-B residents, DMA'd during phase A
        xg_sb = res1.tile([P, NTK, GF], BF16)   # x[:, gsl] chunked by tk
        wo_sb = res1.tile([P, GH, D], BF16)     # Wo[:, gsl].T chunked

        # --- phase A: q/k projections, transposed layout ---
        with (
            tc.tile_pool(name="wqk", bufs=1) as wqkp,
            tc.tile_pool(name="xt", bufs=2) as xtp,
            tc.tile_pool(name="psA", bufs=8, space="PSUM") as psA,
        ):
            wqk_sb = wqkp.tile([P, DK, 2, GF], BF16)
            xts = [xtp.tile([P, DK, TQ], BF16, name="xt") for _ in range(2)]
            ones_sb = const.tile([P, 1], BF16)
            tri_sb = const.tile([P, P], BF16)
            bqk_sb = const.tile([HD, 2, GH], F32)
            cT_sb = const.tile([HD, GH], F32)

            for tci in range(NTQ):
                tsl = slice(tci * TQ, (tci + 1) * TQ)
                xt = xts[tci % 2]
                if tci == 1:
                    # gpsimd library for partition_broadcast: emitted here
                    # so its ~11us DMA never collides with the startup
                    # weight/x transfers; first use is ~90us later
                    nc.gpsimd.load_library(library_config.attn)
                for w, dstT in ((0, qT_sb), (1, kT_sb)):
                    if tci == NTQ - 1:
                        # final tci: one PSUM bank per head with its
                        # activation right after, so the banks drain
                        # incrementally and phase B's first scores never
                        # wait on a multi-activation drain
                        for h in range(GH):
                            psh = psA.tile(
                                [P, TQ], F32, name="psA_t", tag="psA_t"
                            )
                            for dk in range(DK):
                                nc.tensor.matmul(
                                    psh,
                                    wqk_sb[:, dk, w, h * HD : (h + 1) * HD],
                                    xt[:, dk, :],
                                    start=(dk == 0),
                                    stop=(dk == DK - 1),
                                )
                            nc.scalar.activation(
                                dstT[:, h, tsl],
                                psh,
                                mybir.ActivationFunctionType.Identity,
                                bias=bqk_sb[:, w, h : h + 1],
                            )
                        continue
                    ps = [
                        psA.tile([P, TQ], F32, name="psA_t", tag="psA_t")
                        for _ in range(GH)
                    ]
                    for dk in range(DK):
                        if tci == 0 and w == 0:
                            # JIT per-dk DMA issue: each dk's matmuls wait
                            # only on the DMAs issued so far, so the PE
                            # starts after ~2 transfers instead of ~6
                            nc.sync.dma_start(
                                out=wqk_sb[:, dk], in_=wqk[:, dk]
                            )
                            nc.sync.dma_start(
                                out=xts[0][:, dk, :], in_=xT[:, dk, 0:TQ]
                            )
                            if dk == 0:
                                nc.sync.dma_start(out=bqk_sb, in_=bqkT)
                            elif dk == 1:
                                nc.sync.dma_start(out=ones_sb, in_=ones)
                                nc.sync.dma_start(out=tri_sb, in_=tri)
                            elif dk == 2:
                                nc.sync.dma_start(out=cT_sb, in_=cT)
                        for h in range(GH):
                            nc.tensor.matmul(
                                ps[h],
                                wqk_sb[:, dk, w, h * HD : (h + 1) * HD],
                                xt[:, dk, :],
                                start=(dk == 0),
                                stop=(dk == DK - 1),
                            )
                    if tci == 0 and w == 0:
                        # batched prefetches for later phases, issued
                        # behind the critical-path DMAs
                        nc.sync.dma_start(
                            out=xts[1], in_=xT[:, :, TQ : 2 * TQ]
                        )
                        nc.sync.dma_start(out=xg_sb, in_=xg)
                        nc.sync.dma_start(out=wo_sb, in_=woT)
                    if w == 1 and tci + 2 < NTQ:
                        # prefetch next x chunk (single batched issue);
                        # must come after BOTH halves have read xt
                        nsl = slice((tci + 2) * TQ, (tci + 3) * TQ)
                        nc.sync.dma_start(out=xt, in_=xT[:, :, nsl])
                    for h in range(GH):
                        nc.scalar.activation(
                            dstT[:, h, tsl],
                            ps[h],
                            mybir.ActivationFunctionType.Identity,
                            bias=bqk_sb[:, w, h : h + 1],
                        )

        # --- phases B+C interleaved over query chunks ---
        with (
            tc.tile_pool(name="res2", bufs=1) as res2,
            tc.tile_pool(name="wt", bufs=14) as wtp,
            tc.tile_pool(name="rb", bufs=2) as rbp,
            tc.tile_pool(name="pr", bufs=6) as prp,
            tc.tile_pool(name="small", bufs=4) as smp,
            tc.tile_pool(name="stg", bufs=2) as stgp,
            tc.tile_pool(name="psS", bufs=5, space="PSUM") as psS,
            tc.tile_pool(name="psO", bufs=2, space="PSUM") as psO,
            tc.tile_pool(name="psSum", bufs=1, space="PSUM") as psSum,
        ):
            oT_sb = res2.tile([P, GH, T], BF16)   # attention out, transposed

            # Flat software pipeline over every (j, h, i) score unit with a
            # global skew: the exp-hiding lookahead never resets at head or
            # query-chunk boundaries, so the PE sees no dependency stalls
            # there.  Outproj chains for j-1 are injected mid-head.
            sunits = []
            for j in range(NTQ):
                for h in range(GH):
                    for i in range((j + 1) * (TQ // P)):
                        sunits.append((j, h, i))

            st = {}   # (j,h) -> [ps_o, ps_sum, wts, deferred_quads, started]
            pending = None
            stage = None
            QDELAY = 2    # quad colsum deferred this many o-units

            def emit_scores(j, h, i):
                nkk = (j + 1) * (TQ // P)
                g = i - (nkk - TQ // P)
                sub = slice(g * P, TQ) if g >= 0 else slice(0, TQ)
                ps_s = psS.tile([P, TQ], F32, name="ps_s", tag="ps_s")
                nc.tensor.matmul(
                    ps_s[:, sub],
                    kT_sb[:, h, i * P : (i + 1) * P],
                    qT_sb[:, h, j * TQ + sub.start : (j + 1) * TQ],
                    start=True,
                    stop=True,
                )
                wt = wtp.tile([P, TQ], BF16, name="wt")
                nc.scalar.activation(
                    wt[:, sub], ps_s[:, sub],
                    mybir.ActivationFunctionType.Exp,
                    scale=rsqrt_hd,
                )
                if g >= 0:  # triangular mask on leading 128 cols
                    lead = slice(g * P, (g + 1) * P)
                    nc.vector.tensor_mul(wt[:, lead], wt[:, lead], tri_sb)
                if i == 0:
                    st[(j, h)] = [None, None, {}, [], False]
                st[(j, h)][2][i] = (wt, sub)

            def emit_o(j, h, i):
                nonlocal pending, stage
                nkk = (j + 1) * (TQ // P)
                ndiag = TQ // P
                noff = nkk - ndiag
                ent = st[(j, h)]
                # flush the deferred normalize before this head's first
                # ps_sum write (o-unit >= 4 for j >= 1, 0 for j == 0): the
                # single ps_sum bank WAR-waits on its recip, while the
                # deferral keeps the recip chain off the PE critical path
                if pending is not None and i == (0 if j == 0 else 2):
                    _emit_normalize(nc, smp, rbp, oT_sb, cT_sb, mult,
                                    *pending)
                    pending = None
                if i == 0:
                    ent[0] = psO.tile([P, TQ], F32, name="ps_o")
                    ent[1] = psSum.tile([1, TQ], F32, name="ps_sum")
                ps_o, ps_sum, wts = ent[0], ent[1], ent[2]

                def emit_quad(qi):
                    t0 = prp.tile([P, TQ], BF16, name="pr")
                    t1 = prp.tile([P, TQ], BF16, name="pr")
                    q0 = prp.tile([P, TQ], BF16, name="pr")
                    nc.vector.tensor_add(t0, wts[qi - 3][0], wts[qi - 2][0])
                    nc.vector.tensor_add(t1, wts[qi - 1][0], wts[qi][0])
                    nc.vector.tensor_add(q0, t0, t1)
                    nc.tensor.matmul(
                        ps_sum, ones_sb, q0,
                        start=not ent[4], stop=False,
                    )
                    ent[4] = True

                # flush quads that are due (or everything at head end)
                while ent[3] and (ent[3][0][1] <= i or i == nkk - 1):
                    emit_quad(ent[3].pop(0)[0])

                wt, sub = wts[i]
                nc.tensor.matmul(
                    ps_o[:, sub],
                    xg_sb[:, i, h * HD : (h + 1) * HD],
                    wt[:, sub],
                    start=(i == 0),
                    stop=(i == nkk - 1),
                )
                g = i - noff
                if g < 0:
                    if i % 4 == 3:  # off-diagonal quad colsum, deferred
                        ent[3].append((i, i + QDELAY))
                else:
                    # diagonal: per-chunk ones-matmul at its width
                    nc.tensor.matmul(
                        ps_sum[:, sub], ones_sb, wt[:, sub],
                        start=not ent[4],
                        stop=(g == ndiag - 1),
                    )
                    ent[4] = True
                if i == nkk - 1:
                    # head complete: defer our normalize to the next head's
                    # first o-unit
                    pending = (h, j, ps_o, ps_sum)
                    del st[(j, h)]
                if j > 0 and h >= 1 and i == 1:
                    # outproj chains for j-1, spread across heads 1..3
                    lo, hi = [(0, 6), (6, 11), (11, 16)][h - 1]
                    if h == 1:
                        stage = stgp.tile([P, DK, TQ], BF16, name="stage")
                    _emit_outproj(nc, psS, stage, wo_sb, oT_sb, out,
                                  j - 1, lo, hi)

            for u, (j, h, i) in enumerate(sunits):
                emit_scores(j, h, i)
                if u >= SKEW:
                    emit_o(*sunits[u - SKEW])
            for u in range(len(sunits) - SKEW, len(sunits)):
                emit_o(*sunits[u])
            _emit_normalize(nc, smp, rbp, oT_sb, cT_sb, mult, *pending)
            stage = stgp.tile([P, DK, TQ], BF16, name="stage")
            _emit_outproj(nc, psS, stage, wo_sb, oT_sb, out, NTQ - 1, 0, DK)


def _emit_normalize(nc, smp, rbp, oT_sb, cT_sb, mult, h, j, ps_o, ps_sum):
    """1/colsum on one partition, gpsimd partition broadcast, then
    (ps_o * c[p]) * recip in one DVE pass."""
    qsl = slice(j * TQ, (j + 1) * TQ)
    recip = smp.tile([1, TQ], F32, name="recip")
    nc.vector.reciprocal_approx_fast(out=recip, in_=ps_sum)
    rb = rbp.tile([P, TQ], F32, name="rb")
    nc.gpsimd.partition_broadcast(rb, recip)
    nc.vector.scalar_tensor_tensor(
        out=oT_sb[:, h, qsl],
        in0=ps_o,
        scalar=cT_sb[:, h : h + 1],
        in1=rb,
        op0=mult,
        op1=mult,
    )


def _emit_outproj(nc, psS, stage, wo_sb, oT_sb, out, j, lo, hi):
    qsl = slice(j * TQ, (j + 1) * TQ)
    for dk in range(lo, hi):
        ps = psS.tile([P, TQ], F32, name="psC_t", tag="ps_s")
        for m in range(GH):
            nc.tensor.matmul(
                ps,
                wo_sb[:, m, dk * P : (dk + 1) * P],
                oT_sb[:, m, qsl],
                start=(m == 0),
                stop=(m == GH - 1),
            )
        nc.vector.tensor_copy(stage[:, dk, :], ps)
        # flush in groups, smaller at the end so the final drain is short
        if dk in (3, 7, 11, 14, 15):
            flo = {3: 0, 7: 4, 11: 8, 14: 12, 15: 15}[dk]
            nc.sync.dma_start(
                out=out[:, flo : dk + 1, qsl],
                in_=stage[:, flo : dk + 1, :],
            )


def build_program():
    nc = bacc.Bacc(
        "TRN2", target_bir_lowering=False, debug=False, num_devices=NCORES
    )
    f = F32
    # xT: [128, DK, T] bf16 (feature-chunked, feature-on-partition)
    xT = nc.dram_tensor("xT", [P, DK, T], BF16, kind="ExternalInput").ap()
    # xg: [128, NTK, GF] bf16 (time-chunked slice of x for V)
    xg = nc.dram_tensor("xg", [P, NTK, GF], BF16, kind="ExternalInput").ap()
    # wqk: [128, DK, 2, GF] bf16 (fused Wq/Wk, transposed, chunked)
    wqk = nc.dram_tensor(
        "wqk", [P, DK, 2, GF], BF16, kind="ExternalInput"
    ).ap()
    # woT: [128, GH, D] bf16
    woT = nc.dram_tensor("woT", [P, GH, D], BF16, kind="ExternalInput").ap()
    cT = nc.dram_tensor("cT", [HD, GH], f, kind="ExternalInput").ap()
    bqkT = nc.dram_tensor("bqkT", [HD, 2, GH], f, kind="ExternalInput").ap()
    ones = nc.dram_tensor("ones", [P, 1], BF16, kind="ExternalInput").ap()
    tri = nc.dram_tensor("tri", [P, P], BF16, kind="ExternalInput").ap()
    # out: [128, DK, T] bf16 (row-chunked [D, T])
    out = nc.dram_tensor("out", [P, DK, T], BF16, kind="ExternalOutput").ap()

    with tile.TileContext(nc) as tc:
        _body(tc, xT, xg, wqk, woT, cT, bqkT, ones, tri, out)
    nc.compile()
    return nc


_NC_CACHE = None
LAST_RESULT = None
TRACE = False


def kernel(x, Wq, bq, Wk, bk, Wvq, bvq, v_keys, v_embed, Wo, bo):
    global _NC_CACHE, LAST_RESULT
    x = np.asarray(x, np.float32)
    Wq = np.asarray(Wq, np.float32)
    bq = np.asarray(bq, np.float32)
    Wk = np.asarray(Wk, np.float32)
    bk = np.asarray(bk, np.float32)
    v_embed = np.asarray(v_embed, np.float32)
    Wo = np.asarray(Wo, np.float32)
    bo = np.asarray(bo, np.float32)

    c = 2.0 * v_embed[:G].sum(axis=0)
    tri_m = (np.arange(TQ // NTQ)[None, :] >= np.arange(P)[:, None])

    in_maps = []
    for core in range(NCORES):
        b, g = divmod(core, G)
        gsl = slice(g * GF, (g + 1) * GF)
        # [D, X] arrays chunked as [P, D//P, X]: row d -> (d // 128 chunk
        # is INNER on partitions): layout "(n p) x -> p n x"
        xTc = np.ascontiguousarray(
            x[b].T.reshape(DK, P, T).transpose(1, 0, 2)
        ).astype(BF)
        xgc = np.ascontiguousarray(
            x[b][:, gsl].reshape(NTK, P, GF).transpose(1, 0, 2)
        ).astype(BF)
        wq_t = Wq[gsl, :].T  # [D, GF]
        wk_t = Wk[gsl, :].T
        wqk_np = np.stack([wq_t, wk_t], axis=1)  # [D, 2, GF]
        wqkc = np.ascontiguousarray(
            wqk_np.reshape(DK, P, 2, GF).transpose(1, 0, 2, 3)
        ).astype(BF)
        wo_t = Wo[:, gsl].T  # [GF, D]
        woc = np.ascontiguousarray(
            wo_t.reshape(GH, P, D).transpose(1, 0, 2)
        ).astype(BF)
        bqk = np.stack(
            [bq[gsl].reshape(GH, HD).T, bk[gsl].reshape(GH, HD).T], axis=1
        )  # [HD, 2, GH]
        in_maps.append(
            {
                "xT": xTc,
                "xg": xgc,
                "wqk": wqkc,
                "woT": woc,
                "cT": np.ascontiguousarray(c[gsl].reshape(GH, HD).T),
                "bqkT": np.ascontiguousarray(bqk),
                "ones": np.ones((P, 1), BF),
                "tri": tri_m.astype(BF),
            }
        )

    if _NC_CACHE is None:
        _NC_CACHE = build_program()
    res = run_bass_kernel_spmd(
        _NC_CACHE, in_maps, list(range(NCORES)), trace=TRACE
    )
    LAST_RESULT = res

    out = np.zeros((B, T, D), np.float32)
    for core in range(NCORES):
        b = core // G
        # out dram [P, DK, T] -> [D, T] -> [T, D]
        o = res.results[core]["out"].astype(np.float32)
        out[b] += o.transpose(1, 0, 2).reshape(D, T).T
    out += bo[None, None, :]
    return out


if __name__ == "__main__":
    nc = build_program()
    print("built ok")


# revision 34
# speedup vs baseline: 1.0068x; 1.0068x over previous
"""Trainium2 Bass kernel for nn_DMHA_3255585210402 (retrieval_knn DMHA).

Key algebraic fact: TOPK == NVK == 4, so jax.lax.top_k over the size-4 v_keys
axis selects *all* entries; the gather+sum over (DVH, TOPK) reduces to a
constant vector c = 2 * v_embed[0:4].sum(0), and compute_value_states
collapses to  v = x * c.

So the module is causal MHA (B=2, H=16, T=2048, HD=128, D=2048) with
elementwise-scaled V.  Sharding: 8 cores = 2 batches x 4 head-groups.

Final design (~280us HW vs 334us fp32r baseline; rel err 5.1e-3):
  * all matmul operands bf16 (psum stays f32): halves DMA/SBUF, and bf16
    runs 1 cycle/row at ANY moving width (fp32r needs >=256), enabling
    fine-grained causal tiles (diagonal widths 512/384/256/128).
  * triangular mask via DVE tensor_mul with a [128,128] tile (gpsimd
    affine_select was on the exp->o-matmul critical path).
  * softmax denominators: off-diagonal quads summed on DVE then one
    ones-matmul per quad (deferred 2 o-units so the PE never waits on the
    DVE adds); diagonal chunks get per-chunk ones-matmuls at their width.
  * phase B is ONE flat software pipeline over every (j, h, i) score unit
    with a global 4-deep skew (scores pool = 5 psum banks, ps_o 2,
    ps_sum 1): the exp-hiding lookahead never resets at head/chunk
    boundaries.  Normalize (recip -> gpsimd partition_broadcast -> DVE
    scalar_tensor_tensor) is deferred one head and flushed at the next
    head's o-unit 2.
  * outproj chains for j-1 injected mid-head into B(j) (PE-heavy,
    scalar-free work balances the exp-bound stretches); psum->sbuf casts
    on DVE so exp never queues behind them; output staged bf16, flushed
    in 4/4/4/2/1/1-dk DMA groups.
  * DMA: Wq||Wk fused per-dk chunks JIT-issued interleaved with the first
    matmul emissions (sync-engine DMA issue costs ~650ns each, ~2us
    completion latency); everything else as single batched issues; the
    gpsimd library DMA (~11us) deferred past the startup window.
  * phase A: psum groups of 4 banks (q/k separate); final tci uses
    one-bank-per-head chains so banks drain incrementally into phase B.
"""

import math

import numpy as np
import ml_dtypes

import concourse.bass as bass
import concourse.mybir as mybir
import concourse.tile as tile
from concourse import bacc
from concourse.bass_utils import run_bass_kernel_spmd

B, T, D = 2, 2048, 2048
H, HD = 16, 128
G = 4              # head-groups (cores per batch)
GH = H // G        # heads per core
GF = GH * HD       # projected features per core (512)
NCORES = 8
P = 128            # partitions
TQ = 512           # tq chunk width (psum bank / fp32 moving max)
F32 = mybir.dt.float32
F32R = mybir.dt.float32r
BF16 = mybir.dt.bfloat16

DK = D // P        # 16 contraction chunks for projections
NTQ = T // TQ      # 4 query chunks
NTK = T // P       # 16 key chunks
SKEW = 4           # scores-ahead-of-o software pipeline depth

BF = ml_dtypes.bfloat16


def _body(tc, xT, xg, wqk, woT, cT, bqkT, ones, tri, out):
    nc = tc.nc
    rsqrt_hd = 1.0 / math.sqrt(HD)
    mult = mybir.AluOpType.mult

    with (
        tc.tile_pool(name="const", bufs=1) as const,
        tc.tile_pool(name="res1", bufs=1) as res1,
    ):
        from concourse import library_config
        qT_sb = res1.tile([P, GH, T], BF16)   # q, transposed per head
        kT_sb = res1.tile([P, GH, T], BF16)
        # phase-B residents, DMA'd during phase A
        xg_sb = res1.tile([P, NTK, GF], BF16)   # x[:, gsl] chunked by tk
        wo_sb = res1.tile([P, GH, D], BF16)     # Wo[:, gsl].T chunked

        # --- phase A: q/k projections, transposed layout ---
        with (
            tc.tile_pool(name="wqk", bufs=1) as wqkp,
            tc.tile_pool(name="xt", bufs=2) as xtp,
            tc.tile_pool(name="psA", bufs=8, space="PSUM") as psA,
        ):
            wqk_sb = wqkp.tile([P, DK, 2, GF], BF16)
            xts = [xtp.tile([P, DK, TQ], BF16, name="xt") for _ in range(2)]
            ones_sb = const.tile([P, 1], BF16)
            tri_sb = const.tile([P, P], BF16)
            bqk_sb = const.tile([HD, 2, GH], F32)
            cT_sb = const.tile([HD, GH], F32)

            for tci in range(NTQ):
                tsl = slice(tci * TQ, (tci + 1) * TQ)
                xt = xts[tci % 2]
                if tci == 1:
                    # gpsimd library for partition_broadcast: emitted here
                    # so its ~11us DMA never collides with the startup
                    # weight/x transfers; first use is ~90us later
                    nc.gpsimd.load_library(library_config.attn)
                for w, dstT in ((0, qT_sb), (1, kT_sb)):
                    if tci == NTQ - 1:
                        # final tci: one PSUM bank per head with its
                        # activation right after, so the banks drain
                        # incrementally and phase B's first scores never
                        # wait on a multi-activation drain
                        for h in range(GH):
                            psh = psA.tile(
                                [P, TQ], F32, name="psA_t", tag="psA_t"
                            )
                            for dk in range(DK):
                                nc.tensor.matmul(
                                    psh,
                                    wqk_sb[:, dk, w, h * HD : (h + 1) * HD],
                                    xt[:, dk, :],
                                    start=(dk == 0),
                                    stop=(dk == DK - 1),
                                )
                            nc.scalar.activation(
                                dstT[:, h, tsl],
                                psh,
                                mybir.ActivationFunctionType.Identity,
                                bias=bqk_sb[:, w, h : h + 1],
                            )
                        continue
                    ps = [
                        psA.tile([P, TQ], F32, name="psA_t", tag="psA_t")
                        for _ in range(GH)
                    ]
                    for dk in range(DK):
                        if tci == 0 and w == 0:
                            # JIT per-dk DMA issue: each dk's matmuls wait
                            # only on the DMAs issued so far, so the PE
                            # starts after ~2 transfers instead of ~6
                            nc.sync.dma_start(
                                out=wqk_sb[:, dk], in_=wqk[:, dk]
                            )
                            nc.sync.dma_start(
                                out=xts[0][:, dk, :], in_=xT[:, dk, 0:TQ]
                            )
                            if dk == 0:
                                nc.sync.dma_start(out=bqk_sb, in_=bqkT)
                            elif dk == 1:
                                nc.sync.dma_start(out=ones_sb, in_=ones)
                                nc.sync.dma_start(out=tri_sb, in_=tri)
                            elif dk == 2:
                                nc.sync.dma_start(out=cT_sb, in_=cT)
                        for h in range(GH):
                            nc.tensor.matmul(
                                ps[h],
                                wqk_sb[:, dk, w, h * HD : (h + 1) * HD],
                                xt[:, dk, :],
                                start=(dk == 0),
                                stop=(dk == DK - 1),
                            )
                    if tci == 0 and w == 0:
                        # batched prefetches for later phases, issued
                        # behind the critical-path DMAs
                        nc.sync.dma_start(
                            out=xts[1], in_=xT[:, :, TQ : 2 * TQ]
                        )
                        nc.sync.dma_start(out=xg_sb, in_=xg)
                        nc.sync.dma_start(out=wo_sb, in_=woT)
                    if w == 1 and tci + 2 < NTQ:
                        # prefetch next x chunk (single batched issue);
                        # must come after BOTH halves have read xt
                        nsl = slice((tci + 2) * TQ, (tci + 3) * TQ)
                        nc.sync.dma_start(out=xt, in_=xT[:, :, nsl])
                    for h in range(GH):
                        nc.scalar.activation(
                            dstT[:, h, tsl],
                            ps[h],
                            mybir.ActivationFunctionType.Identity,
                            bias=bqk_sb[:, w, h : h + 1],
                        )

        # --- phases B+C interleaved over query chunks ---
        with (
            tc.tile_pool(name="res2", bufs=1) as res2,
            tc.tile_pool(name="wt", bufs=14) as wtp,
            tc.tile_pool(name="rb", bufs=2) as rbp,
            tc.tile_pool(name="pr", bufs=6) as prp,
            tc.tile_pool(name="small", bufs=4) as smp,
            tc.tile_pool(name="stg", bufs=2) as stgp,
            tc.tile_pool(name="psS", bufs=5, space="PSUM") as psS,
            tc.tile_pool(name="psO", bufs=2, space="PSUM") as psO,
            tc.tile_pool(name="psSum", bufs=1, space="PSUM") as psSum,
        ):
            oT_sb = res2.tile([P, GH, T], BF16)   # attention out, transposed

            # Flat software pipeline over every (j, h, i) score unit with a
            # global skew: the exp-hiding lookahead never resets at head or
            # query-chunk boundaries, so the PE sees no dependency stalls
            # there.  Outproj chains for j-1 are injected mid-head.
            sunits = []
            for j in range(NTQ):
                for h in range(GH):
                    for i in range((j + 1) * (TQ // P)):
                        sunits.append((j, h, i))

            st = {}   # (j,h) -> [ps_o, ps_sum, wts, deferred_quads, started]
            pending = None
            stage = None
            QDELAY = 2    # quad colsum deferred this many o-units

            def emit_scores(j, h, i):
                nkk = (j + 1) * (TQ // P)
                g = i - (nkk - TQ // P)
                sub = slice(g * P, TQ) if g >= 0 else slice(0, TQ)
                ps_s = psS.tile([P, TQ], F32, name="ps_s", tag="ps_s")
                nc.tensor.matmul(
                    ps_s[:, sub],
                    kT_sb[:, h, i * P : (i + 1) * P],
                    qT_sb[:, h, j * TQ + sub.start : (j + 1) * TQ],
                    start=True,
                    stop=True,
                )
                wt = wtp.tile([P, TQ], BF16, name="wt")
                nc.scalar.activation(
                    wt[:, sub], ps_s[:, sub],
                    mybir.ActivationFunctionType.Exp,
                    scale=rsqrt_hd,
                )
                if g >= 0:  # triangular mask on leading 128 cols
                    lead = slice(g * P, (g + 1) * P)
                    nc.vector.tensor_mul(wt[:, lead], wt[:, lead], tri_sb)
                if i == 0:
                    st[(j, h)] = [None, None, {}, [], False]
                st[(j, h)][2][i] = (wt, sub)

            def emit_o(j, h, i):
                nonlocal pending, stage
                nkk = (j + 1) * (TQ // P)
                ndiag = TQ // P
                noff = nkk - ndiag
                ent = st[(j, h)]
                # flush the deferred normalize before this head's first
                # ps_sum write (o-unit >= 4 for j >= 1, 0 for j == 0): the
                # single ps_sum bank WAR-waits on its recip, while the
                # deferral keeps the recip chain off the PE critical path
                if pending is not None and i == (0 if j == 0 else 1):
                    _emit_normalize(nc, smp, rbp, oT_sb, cT_sb, mult,
                                    *pending)
                    pending = None
                if i == 0:
                    ent[0] = psO.tile([P, TQ], F32, name="ps_o")
                    ent[1] = psSum.tile([1, TQ], F32, name="ps_sum")
                ps_o, ps_sum, wts = ent[0], ent[1], ent[2]

                def emit_quad(qi):
                    t0 = prp.tile([P, TQ], BF16, name="pr")
                    t1 = prp.tile([P, TQ], BF16, name="pr")
                    q0 = prp.tile([P, TQ], BF16, name="pr")
                    nc.vector.tensor_add(t0, wts[qi - 3][0], wts[qi - 2][0])
                    nc.vector.tensor_add(t1, wts[qi - 1][0], wts[qi][0])
                    nc.vector.tensor_add(q0, t0, t1)
                    nc.tensor.matmul(
                        ps_sum, ones_sb, q0,
                        start=not ent[4], stop=False,
                    )
                    ent[4] = True

                # flush quads that are due (or everything at head end)
                while ent[3] and (ent[3][0][1] <= i or i == nkk - 1):
                    emit_quad(ent[3].pop(0)[0])

                wt, sub = wts[i]
                nc.tensor.matmul(
                    ps_o[:, sub],
                    xg_sb[:, i, h * HD : (h + 1) * HD],
                    wt[:, sub],
                    start=(i == 0),
                    stop=(i == nkk - 1),
                )
                g = i - noff
                if g < 0:
                    if i % 4 == 3:  # off-diagonal quad colsum, deferred
                        ent[3].append((i, i + QDELAY))
                else:
                    # diagonal: per-chunk ones-matmul at its width
                    nc.tensor.matmul(
                        ps_sum[:, sub], ones_sb, wt[:, sub],
                        start=not ent[4],
                        stop=(g == ndiag - 1),
                    )
                    ent[4] = True
                if i == nkk - 1:
                    # head complete: defer our normalize to the next head's
                    # first o-unit
                    pending = (h, j, ps_o, ps_sum)
                    del st[(j, h)]
                if j > 0 and h >= 1 and i in (1, 3 + nkk // 4):
                    # outproj chains for j-1, spread across heads 1..3 and
                    # two injection points per head (smaller bursts keep
                    # the shared scores-psum pool from backing up)
                    lo, hi = [(0, 6), (6, 11), (11, 16)][h - 1]
                    mid = (lo + hi + 1) // 2
                    if i == 1:
                        if h == 1:
                            stage = stgp.tile(
                                [P, DK, TQ], BF16, name="stage"
                            )
                        _emit_outproj(nc, psS, stage, wo_sb, oT_sb, out,
                                      j - 1, lo, mid)
                    else:
                        _emit_outproj(nc, psS, stage, wo_sb, oT_sb, out,
                                      j - 1, mid, hi)

            for u, (j, h, i) in enumerate(sunits):
                emit_scores(j, h, i)
                if u >= SKEW:
                    emit_o(*sunits[u - SKEW])
            for u in range(len(sunits) - SKEW, len(sunits)):
                emit_o(*sunits[u])
            _emit_normalize(nc, smp, rbp, oT_sb, cT_sb, mult, *pending)
            stage = stgp.tile([P, DK, TQ], BF16, name="stage")
            _emit_outproj(nc, psS, stage, wo_sb, oT_sb, out, NTQ - 1, 0, DK)


def _emit_normalize(nc, smp, rbp, oT_sb, cT_sb, mult, h, j, ps_o, ps_sum):
    """1/colsum on one partition, gpsimd partition broadcast, then
    (ps_o * c[p]) * recip in one DVE pass."""
    qsl = slice(j * TQ, (j + 1) * TQ)
    recip = smp.tile([1, TQ], F32, name="recip")
    nc.vector.reciprocal_approx_fast(out=recip, in_=ps_sum)
    rb = rbp.tile([P, TQ], F32, name="rb")
    nc.gpsimd.partition_broadcast(rb, recip)
    nc.vector.scalar_tensor_tensor(
        out=oT_sb[:, h, qsl],
        in0=ps_o,
        scalar=cT_sb[:, h : h + 1],
        in1=rb,
        op0=mult,
        op1=mult,
    )


def _emit_outproj(nc, psS, stage, wo_sb, oT_sb, out, j, lo, hi):
    qsl = slice(j * TQ, (j + 1) * TQ)
    for dk in range(lo, hi):
        ps = psS.tile([P, TQ], F32, name="psC_t", tag="ps_s")
        for m in range(GH):
            nc.tensor.matmul(
                ps,
                wo_sb[:, m, dk * P : (dk + 1) * P],
                oT_sb[:, m, qsl],
                start=(m == 0),
                stop=(m == GH - 1),
            )
        nc.vector.tensor_copy(stage[:, dk, :], ps)
        # flush in groups, smaller at the end so the final drain is short
        if dk in (3, 7, 11, 14, 15):
            flo = {3: 0, 7: 4, 11: 8, 14: 12, 15: 15}[dk]
            nc.sync.dma_start(
                out=out[:, flo : dk + 1, qsl],
                in_=stage[:, flo : dk + 1, :],
            )


def build_program():
    nc = bacc.Bacc(
        "TRN2", target_bir_lowering=False, debug=False, num_devices=NCORES
    )
    f = F32
    # xT: [128, DK, T] bf16 (feature-chunked, feature-on-partition)
    xT = nc.dram_tensor("xT", [P, DK, T], BF16, kind="ExternalInput").ap()
    # xg: [128, NTK, GF] bf16 (time-chunked slice of x for V)
    xg = nc.dram_tensor("xg", [P, NTK, GF], BF16, kind="ExternalInput").ap()
    # wqk: [128, DK, 2, GF] bf16 (fused Wq/Wk, transposed, chunked)
    wqk = nc.dram_tensor(
        "wqk", [P, DK, 2, GF], BF16, kind="ExternalInput"
    ).ap()
    # woT: [128, GH, D] bf16
    woT = nc.dram_tensor("woT", [P, GH, D], BF16, kind="ExternalInput").ap()
    cT = nc.dram_tensor("cT", [HD, GH], f, kind="ExternalInput").ap()
    bqkT = nc.dram_tensor("bqkT", [HD, 2, GH], f, kind="ExternalInput").ap()
    ones = nc.dram_tensor("ones", [P, 1], BF16, kind="ExternalInput").ap()
    tri = nc.dram_tensor("tri", [P, P], BF16, kind="ExternalInput").ap()
    # out: [128, DK, T] bf16 (row-chunked [D, T])
    out = nc.dram_tensor("out", [P, DK, T], BF16, kind="ExternalOutput").ap()

    with tile.TileContext(nc) as tc:
        _body(tc, xT, xg, wqk, woT, cT, bqkT, ones, tri, out)
    nc.compile()
    return nc


_NC_CACHE = None
LAST_RESULT = None
TRACE = False


def kernel(x, Wq, bq, Wk, bk, Wvq, bvq, v_keys, v_embed, Wo, bo):
    global _NC_CACHE, LAST_RESULT
    x = np.asarray(x, np.float32)
    Wq = np.asarray(Wq, np.float32)
    bq = np.asarray(bq, np.float32)
    Wk = np.asarray(Wk, np.float32)
    bk = np.asarray(bk, np.float32)
    v_embed = np.asarray(v_embed, np.float32)
    Wo = np.asarray(Wo, np.float32)
    bo = np.asarray(bo, np.float32)

    c = 2.0 * v_embed[:G].sum(axis=0)
    tri_m = (np.arange(TQ // NTQ)[None, :] >= np.arange(P)[:, None])

    in_maps = []
    for core in range(NCORES):
        b, g = divmod(core, G)
        gsl = slice(g * GF, (g + 1) * GF)
        # [D, X] arrays chunked as [P, D//P, X]: row d -> (d // 128 chunk
        # is INNER on partitions): layout "(n p) x -> p n x"
        xTc = np.ascontiguousarray(
            x[b].T.reshape(DK, P, T).transpose(1, 0, 2)
        ).astype(BF)
        xgc = np.ascontiguousarray(
            x[b][:, gsl].reshape(NTK, P, GF).transpose(1, 0, 2)
        ).astype(BF)
        wq_t = Wq[gsl, :].T  # [D, GF]
        wk_t = Wk[gsl, :].T
        wqk_np = np.stack([wq_t, wk_t], axis=1)  # [D, 2, GF]
        wqkc = np.ascontiguousarray(
            wqk_np.reshape(DK, P, 2, GF).transpose(1, 0, 2, 3)
        ).astype(BF)
        wo_t = Wo[:, gsl].T  # [GF, D]
        woc = np.ascontiguousarray(
            wo_t.reshape(GH, P, D).transpose(1, 0, 2)
        ).astype(BF)
        bqk = np.stack(
            [bq[gsl].reshape(GH, HD).T, bk[gsl].reshape(GH, HD).T], axis=1
        )  # [HD, 2, GH]
        in_maps.append(
            {
                "xT": xTc,
                "xg": xgc,
                "wqk": wqkc,
                "woT": woc,
                "cT": np.ascontiguousarray(c[gsl].reshape(GH, HD).T),
                "bqkT": np.ascontiguousarray(bqk),
                "ones": np.ones((P, 1), BF),
                "tri": tri_m.astype(BF),
            }
        )

    if _NC_CACHE is None:
        _NC_CACHE = build_program()
    res = run_bass_kernel_spmd(
        _NC_CACHE, in_maps, list(range(NCORES)), trace=TRACE
    )
    LAST_RESULT = res

    out = np.zeros((B, T, D), np.float32)
    for core in range(NCORES):
        b = core // G
        # out dram [P, DK, T] -> [D, T] -> [T, D]
        o = res.results[core]["out"].astype(np.float32)
        out[b] += o.transpose(1, 0, 2).reshape(D, T).T
    out += bo[None, None, :]
    return out


if __name__ == "__main__":
    nc = build_program()
    print("built ok")


# revision 35
# speedup vs baseline: 1.0146x; 1.0078x over previous
"""Trainium2 Bass kernel for nn_DMHA_3255585210402 (retrieval_knn DMHA).

Key algebraic fact: TOPK == NVK == 4, so jax.lax.top_k over the size-4 v_keys
axis selects *all* entries; the gather+sum over (DVH, TOPK) reduces to a
constant vector c = 2 * v_embed[0:4].sum(0), and compute_value_states
collapses to  v = x * c.

So the module is causal MHA (B=2, H=16, T=2048, HD=128, D=2048) with
elementwise-scaled V.  Sharding: 8 cores = 2 batches x 4 head-groups.

Final design (~280us HW vs 334us fp32r baseline; rel err 5.1e-3):
  * all matmul operands bf16 (psum stays f32): halves DMA/SBUF, and bf16
    runs 1 cycle/row at ANY moving width (fp32r needs >=256), enabling
    fine-grained causal tiles (diagonal widths 512/384/256/128).
  * triangular mask via DVE tensor_mul with a [128,128] tile (gpsimd
    affine_select was on the exp->o-matmul critical path).
  * softmax denominators: off-diagonal quads summed on DVE then one
    ones-matmul per quad (deferred 2 o-units so the PE never waits on the
    DVE adds); diagonal chunks get per-chunk ones-matmuls at their width.
  * phase B is ONE flat software pipeline over every (j, h, i) score unit
    with a global 4-deep skew (scores pool = 5 psum banks, ps_o 2,
    ps_sum 1): the exp-hiding lookahead never resets at head/chunk
    boundaries.  Normalize (recip -> gpsimd partition_broadcast -> DVE
    scalar_tensor_tensor) is deferred one head and flushed at the next
    head's o-unit 2.
  * outproj chains for j-1 injected mid-head into B(j) (PE-heavy,
    scalar-free work balances the exp-bound stretches); psum->sbuf casts
    on DVE so exp never queues behind them; output staged bf16, flushed
    in 4/4/4/2/1/1-dk DMA groups.
  * DMA: Wq||Wk fused per-dk chunks JIT-issued interleaved with the first
    matmul emissions (sync-engine DMA issue costs ~650ns each, ~2us
    completion latency); everything else as single batched issues; the
    gpsimd library DMA (~11us) deferred past the startup window.
  * phase A: psum groups of 4 banks (q/k separate); final tci uses
    one-bank-per-head chains so banks drain incrementally into phase B.
"""

import math

import numpy as np
import ml_dtypes

import concourse.bass as bass
import concourse.mybir as mybir
import concourse.tile as tile
from concourse import bacc
from concourse.bass_utils import run_bass_kernel_spmd

B, T, D = 2, 2048, 2048
H, HD = 16, 128
G = 4              # head-groups (cores per batch)
GH = H // G        # heads per core
GF = GH * HD       # projected features per core (512)
NCORES = 8
P = 128            # partitions
TQ = 512           # tq chunk width (psum bank / fp32 moving max)
F32 = mybir.dt.float32
F32R = mybir.dt.float32r
BF16 = mybir.dt.bfloat16

DK = D // P        # 16 contraction chunks for projections
NTQ = T // TQ      # 4 query chunks
NTK = T // P       # 16 key chunks
SKEW = 4           # scores-ahead-of-o software pipeline depth

BF = ml_dtypes.bfloat16


def _body(tc, xT, xg, wqk, woT, cT, bqkT, ones, tri, out):
    nc = tc.nc
    rsqrt_hd = 1.0 / math.sqrt(HD)
    mult = mybir.AluOpType.mult

    with (
        tc.tile_pool(name="const", bufs=1) as const,
        tc.tile_pool(name="res1", bufs=1) as res1,
    ):
        from concourse import library_config
        qT_sb = res1.tile([P, GH, T], BF16)   # q, transposed per head
        kT_sb = res1.tile([P, GH, T], BF16)
        # phase-B residents, DMA'd during phase A
        xg_sb = res1.tile([P, NTK, GF], BF16)   # x[:, gsl] chunked by tk
        wo_sb = res1.tile([P, GH, D], BF16)     # Wo[:, gsl].T chunked

        # --- phase A: q/k projections, transposed layout ---
        with (
            tc.tile_pool(name="wqk", bufs=1) as wqkp,
            tc.tile_pool(name="xt", bufs=2) as xtp,
            tc.tile_pool(name="psA", bufs=8, space="PSUM") as psA,
        ):
            wqk_sb = wqkp.tile([P, DK, 2, GF], BF16)
            xts = [xtp.tile([P, DK, TQ], BF16, name="xt") for _ in range(2)]
            ones_sb = const.tile([P, 1], BF16)
            tri_sb = const.tile([P, P], BF16)
            bqk_sb = const.tile([HD, 2, GH], F32)
            cT_sb = const.tile([HD, GH], F32)

            for tci in range(NTQ):
                tsl = slice(tci * TQ, (tci + 1) * TQ)
                xt = xts[tci % 2]
                if tci == 1:
                    # gpsimd library for partition_broadcast: emitted here
                    # so its ~11us DMA never collides with the startup
                    # weight/x transfers; first use is ~90us later
                    nc.gpsimd.load_library(library_config.attn)
                for w, dstT in ((0, qT_sb), (1, kT_sb)):
                    if tci == NTQ - 1:
                        # final tci: one PSUM bank per head with its
                        # activation right after, so the banks drain
                        # incrementally and phase B's first scores never
                        # wait on a multi-activation drain
                        for h in range(GH):
                            psh = psA.tile(
                                [P, TQ], F32, name="psA_t", tag="psA_t"
                            )
                            for dk in range(DK):
                                nc.tensor.matmul(
                                    psh,
                                    wqk_sb[:, dk, w, h * HD : (h + 1) * HD],
                                    xt[:, dk, :],
                                    start=(dk == 0),
                                    stop=(dk == DK - 1),
                                )
                            nc.scalar.activation(
                                dstT[:, h, tsl],
                                psh,
                                mybir.ActivationFunctionType.Identity,
                                bias=bqk_sb[:, w, h : h + 1],
                            )
                        continue
                    ps = [
                        psA.tile([P, TQ], F32, name="psA_t", tag="psA_t")
                        for _ in range(GH)
                    ]
                    for dk in range(DK):
                        if tci == 0 and w == 0:
                            # JIT per-dk DMA issue: each dk's matmuls wait
                            # only on the DMAs issued so far, so the PE
                            # starts after ~2 transfers instead of ~6
                            nc.sync.dma_start(
                                out=wqk_sb[:, dk], in_=wqk[:, dk]
                            )
                            nc.sync.dma_start(
                                out=xts[0][:, dk, :], in_=xT[:, dk, 0:TQ]
                            )
                            if dk == 0:
                                nc.sync.dma_start(out=bqk_sb, in_=bqkT)
                            elif dk == 1:
                                nc.sync.dma_start(out=ones_sb, in_=ones)
                                nc.sync.dma_start(out=tri_sb, in_=tri)
                            elif dk == 2:
                                nc.sync.dma_start(out=cT_sb, in_=cT)
                        for h in range(GH):
                            nc.tensor.matmul(
                                ps[h],
                                wqk_sb[:, dk, w, h * HD : (h + 1) * HD],
                                xt[:, dk, :],
                                start=(dk == 0),
                                stop=(dk == DK - 1),
                            )
                    if tci == 0 and w == 0:
                        # batched prefetches for later phases, issued
                        # behind the critical-path DMAs
                        nc.sync.dma_start(
                            out=xts[1], in_=xT[:, :, TQ : 2 * TQ]
                        )
                        nc.sync.dma_start(out=xg_sb, in_=xg)
                        nc.sync.dma_start(out=wo_sb, in_=woT)
                    if w == 1 and tci + 2 < NTQ:
                        # prefetch next x chunk (single batched issue);
                        # must come after BOTH halves have read xt
                        nsl = slice((tci + 2) * TQ, (tci + 3) * TQ)
                        nc.sync.dma_start(out=xt, in_=xT[:, :, nsl])
                    for h in range(GH):
                        nc.scalar.activation(
                            dstT[:, h, tsl],
                            ps[h],
                            mybir.ActivationFunctionType.Identity,
                            bias=bqk_sb[:, w, h : h + 1],
                        )

        # --- phases B+C interleaved over query chunks ---
        with (
            tc.tile_pool(name="res2", bufs=1) as res2,
            tc.tile_pool(name="wt", bufs=14) as wtp,
            tc.tile_pool(name="rb", bufs=2) as rbp,
            tc.tile_pool(name="pr", bufs=6) as prp,
            tc.tile_pool(name="small", bufs=4) as smp,
            tc.tile_pool(name="stg", bufs=2) as stgp,
            tc.tile_pool(name="psS", bufs=5, space="PSUM") as psS,
            tc.tile_pool(name="psO", bufs=2, space="PSUM") as psO,
            tc.tile_pool(name="psSum", bufs=1, space="PSUM") as psSum,
        ):
            oT_sb = res2.tile([P, GH, T], BF16)   # attention out, transposed

            # Flat software pipeline over every (j, h, i) score unit with a
            # global skew: the exp-hiding lookahead never resets at head or
            # query-chunk boundaries, so the PE sees no dependency stalls
            # there.  Outproj chains for j-1 are injected mid-head.
            sunits = []
            for j in range(NTQ):
                for h in range(GH):
                    for i in range((j + 1) * (TQ // P)):
                        sunits.append((j, h, i))

            st = {}   # (j,h) -> [ps_o, ps_sum, wts, deferred_quads, started]
            pending = None
            stage = None
            QDELAY = 2    # quad colsum deferred this many o-units

            def emit_scores(j, h, i):
                nkk = (j + 1) * (TQ // P)
                g = i - (nkk - TQ // P)
                sub = slice(g * P, TQ) if g >= 0 else slice(0, TQ)
                ps_s = psS.tile([P, TQ], F32, name="ps_s", tag="ps_s")
                nc.tensor.matmul(
                    ps_s[:, sub],
                    kT_sb[:, h, i * P : (i + 1) * P],
                    qT_sb[:, h, j * TQ + sub.start : (j + 1) * TQ],
                    start=True,
                    stop=True,
                )
                wt = wtp.tile([P, TQ], BF16, name="wt")
                nc.scalar.activation(
                    wt[:, sub], ps_s[:, sub],
                    mybir.ActivationFunctionType.Exp,
                    scale=rsqrt_hd,
                )
                if g >= 0:  # triangular mask on leading 128 cols
                    lead = slice(g * P, (g + 1) * P)
                    nc.vector.tensor_mul(wt[:, lead], wt[:, lead], tri_sb)
                    if g >= 1:  # zero invalid prefix for the diag quad
                        nc.vector.memset(wt[:, 0 : g * P], 0.0)
                if i == 0:
                    st[(j, h)] = [None, None, {}, [], False]
                st[(j, h)][2][i] = (wt, sub)

            def emit_o(j, h, i):
                nonlocal pending, stage
                nkk = (j + 1) * (TQ // P)
                ndiag = TQ // P
                noff = nkk - ndiag
                ent = st[(j, h)]
                # flush the deferred normalize before this head's first
                # ps_sum write (o-unit >= 4 for j >= 1, 0 for j == 0): the
                # single ps_sum bank WAR-waits on its recip, while the
                # deferral keeps the recip chain off the PE critical path
                if pending is not None and i == (0 if j == 0 else 1):
                    _emit_normalize(nc, smp, rbp, oT_sb, cT_sb, mult,
                                    *pending)
                    pending = None
                if i == 0:
                    ent[0] = psO.tile([P, TQ], F32, name="ps_o")
                    ent[1] = psSum.tile([1, TQ], F32, name="ps_sum")
                ps_o, ps_sum, wts = ent[0], ent[1], ent[2]

                def emit_quad(qi):
                    t0 = prp.tile([P, TQ], BF16, name="pr")
                    t1 = prp.tile([P, TQ], BF16, name="pr")
                    q0 = prp.tile([P, TQ], BF16, name="pr")
                    nc.vector.tensor_add(t0, wts[qi - 3][0], wts[qi - 2][0])
                    nc.vector.tensor_add(t1, wts[qi - 1][0], wts[qi][0])
                    nc.vector.tensor_add(q0, t0, t1)
                    nc.tensor.matmul(
                        ps_sum, ones_sb, q0,
                        start=not ent[4], stop=False,
                    )
                    ent[4] = True

                # flush quads that are due (or everything at head end)
                while ent[3] and (ent[3][0][1] <= i or i == nkk - 1):
                    emit_quad(ent[3].pop(0)[0])

                wt, sub = wts[i]
                nc.tensor.matmul(
                    ps_o[:, sub],
                    xg_sb[:, i, h * HD : (h + 1) * HD],
                    wt[:, sub],
                    start=(i == 0),
                    stop=(i == nkk - 1),
                )
                g = i - noff
                if g < 0:
                    if i % 4 == 3:  # off-diagonal quad colsum, deferred
                        ent[3].append((i, i + QDELAY))
                elif g == ndiag - 1:
                    # diagonal colsum: single full-width quad (prefixes
                    # zeroed above) replaces 4 narrow ones-matmuls
                    t0 = prp.tile([P, TQ], BF16, name="pr")
                    t1 = prp.tile([P, TQ], BF16, name="pr")
                    q0 = prp.tile([P, TQ], BF16, name="pr")
                    nc.vector.tensor_add(t0, wts[i - 3][0], wts[i - 2][0])
                    nc.vector.tensor_add(t1, wts[i - 1][0], wts[i][0])
                    nc.vector.tensor_add(q0, t0, t1)
                    nc.tensor.matmul(
                        ps_sum, ones_sb, q0,
                        start=not ent[4], stop=True,
                    )
                    ent[4] = True
                if i == nkk - 1:
                    # head complete: defer our normalize to the next head's
                    # first o-unit
                    pending = (h, j, ps_o, ps_sum)
                    del st[(j, h)]
                if j > 0 and h >= 1 and i in (1, 3 + nkk // 4):
                    # outproj chains for j-1, spread across heads 1..3 and
                    # two injection points per head (smaller bursts keep
                    # the shared scores-psum pool from backing up)
                    lo, hi = [(0, 6), (6, 11), (11, 16)][h - 1]
                    mid = (lo + hi + 1) // 2
                    if i == 1:
                        if h == 1:
                            stage = stgp.tile(
                                [P, DK, TQ], BF16, name="stage"
                            )
                        _emit_outproj(nc, psS, stage, wo_sb, oT_sb, out,
                                      j - 1, lo, mid)
                    else:
                        _emit_outproj(nc, psS, stage, wo_sb, oT_sb, out,
                                      j - 1, mid, hi)

            for u, (j, h, i) in enumerate(sunits):
                emit_scores(j, h, i)
                if u >= SKEW:
                    emit_o(*sunits[u - SKEW])
            for u in range(len(sunits) - SKEW, len(sunits)):
                emit_o(*sunits[u])
            _emit_normalize(nc, smp, rbp, oT_sb, cT_sb, mult, *pending)
            stage = stgp.tile([P, DK, TQ], BF16, name="stage")
            _emit_outproj(nc, psS, stage, wo_sb, oT_sb, out, NTQ - 1, 0, DK)


def _emit_normalize(nc, smp, rbp, oT_sb, cT_sb, mult, h, j, ps_o, ps_sum):
    """1/colsum on one partition, gpsimd partition broadcast, then
    (ps_o * c[p]) * recip in one DVE pass."""
    qsl = slice(j * TQ, (j + 1) * TQ)
    recip = smp.tile([1, TQ], F32, name="recip")
    nc.vector.reciprocal_approx_fast(out=recip, in_=ps_sum)
    rb = rbp.tile([P, TQ], F32, name="rb")
    nc.gpsimd.partition_broadcast(rb, recip)
    nc.vector.scalar_tensor_tensor(
        out=oT_sb[:, h, qsl],
        in0=ps_o,
        scalar=cT_sb[:, h : h + 1],
        in1=rb,
        op0=mult,
        op1=mult,
    )


def _emit_outproj(nc, psS, stage, wo_sb, oT_sb, out, j, lo, hi):
    qsl = slice(j * TQ, (j + 1) * TQ)
    for dk in range(lo, hi):
        ps = psS.tile([P, TQ], F32, name="psC_t", tag="ps_s")
        for m in range(GH):
            nc.tensor.matmul(
                ps,
                wo_sb[:, m, dk * P : (dk + 1) * P],
                oT_sb[:, m, qsl],
                start=(m == 0),
                stop=(m == GH - 1),
            )
        nc.vector.tensor_copy(stage[:, dk, :], ps)
        # flush in groups, smaller at the end so the final drain is short
        if dk in (3, 7, 11, 14, 15):
            flo = {3: 0, 7: 4, 11: 8, 14: 12, 15: 15}[dk]
            nc.sync.dma_start(
                out=out[:, flo : dk + 1, qsl],
                in_=stage[:, flo : dk + 1, :],
            )


def build_program():
    nc = bacc.Bacc(
        "TRN2", target_bir_lowering=False, debug=False, num_devices=NCORES
    )
    f = F32
    # xT: [128, DK, T] bf16 (feature-chunked, feature-on-partition)
    xT = nc.dram_tensor("xT", [P, DK, T], BF16, kind="ExternalInput").ap()
    # xg: [128, NTK, GF] bf16 (time-chunked slice of x for V)
    xg = nc.dram_tensor("xg", [P, NTK, GF], BF16, kind="ExternalInput").ap()
    # wqk: [128, DK, 2, GF] bf16 (fused Wq/Wk, transposed, chunked)
    wqk = nc.dram_tensor(
        "wqk", [P, DK, 2, GF], BF16, kind="ExternalInput"
    ).ap()
    # woT: [128, GH, D] bf16
    woT = nc.dram_tensor("woT", [P, GH, D], BF16, kind="ExternalInput").ap()
    cT = nc.dram_tensor("cT", [HD, GH], f, kind="ExternalInput").ap()
    bqkT = nc.dram_tensor("bqkT", [HD, 2, GH], f, kind="ExternalInput").ap()
    ones = nc.dram_tensor("ones", [P, 1], BF16, kind="ExternalInput").ap()
    tri = nc.dram_tensor("tri", [P, P], BF16, kind="ExternalInput").ap()
    # out: [128, DK, T] bf16 (row-chunked [D, T])
    out = nc.dram_tensor("out", [P, DK, T], BF16, kind="ExternalOutput").ap()

    with tile.TileContext(nc) as tc:
        _body(tc, xT, xg, wqk, woT, cT, bqkT, ones, tri, out)
    nc.compile()
    return nc


_NC_CACHE = None
LAST_RESULT = None
TRACE = False


def kernel(x, Wq, bq, Wk, bk, Wvq, bvq, v_keys, v_embed, Wo, bo):
    global _NC_CACHE, LAST_RESULT
    x = np.asarray(x, np.float32)
    Wq = np.asarray(Wq, np.float32)
    bq = np.asarray(bq, np.float32)
    Wk = np.asarray(Wk, np.float32)
    bk = np.asarray(bk, np.float32)
    v_embed = np.asarray(v_embed, np.float32)
    Wo = np.asarray(Wo, np.float32)
    bo = np.asarray(bo, np.float32)

    c = 2.0 * v_embed[:G].sum(axis=0)
    tri_m = (np.arange(TQ // NTQ)[None, :] >= np.arange(P)[:, None])

    in_maps = []
    for core in range(NCORES):
        b, g = divmod(core, G)
        gsl = slice(g * GF, (g + 1) * GF)
        # [D, X] arrays chunked as [P, D//P, X]: row d -> (d // 128 chunk
        # is INNER on partitions): layout "(n p) x -> p n x"
        xTc = np.ascontiguousarray(
            x[b].T.reshape(DK, P, T).transpose(1, 0, 2)
        ).astype(BF)
        xgc = np.ascontiguousarray(
            x[b][:, gsl].reshape(NTK, P, GF).transpose(1, 0, 2)
        ).astype(BF)
        wq_t = Wq[gsl, :].T  # [D, GF]
        wk_t = Wk[gsl, :].T
        wqk_np = np.stack([wq_t, wk_t], axis=1)  # [D, 2, GF]
        wqkc = np.ascontiguousarray(
            wqk_np.reshape(DK, P, 2, GF).transpose(1, 0, 2, 3)
        ).astype(BF)
        wo_t = Wo[:, gsl].T  # [GF, D]
        woc = np.ascontiguousarray(
            wo_t.reshape(GH, P, D).transpose(1, 0, 2)
        ).astype(BF)
        bqk = np.stack(
            [bq[gsl].reshape(GH, HD).T, bk[gsl].reshape(GH, HD).T], axis=1
        )  # [HD, 2, GH]
        in_maps.append(
            {
                "xT": xTc,
                "xg": xgc,
                "wqk": wqkc,
                "woT": woc,
                "cT": np.ascontiguousarray(c[gsl].reshape(GH, HD).T),
                "bqkT": np.ascontiguousarray(bqk),
                "ones": np.ones((P, 1), BF),
                "tri": tri_m.astype(BF),
            }
        )

    if _NC_CACHE is None:
        _NC_CACHE = build_program()
    res = run_bass_kernel_spmd(
        _NC_CACHE, in_maps, list(range(NCORES)), trace=TRACE
    )
    LAST_RESULT = res

    out = np.zeros((B, T, D), np.float32)
    for core in range(NCORES):
        b = core // G
        # out dram [P, DK, T] -> [D, T] -> [T, D]
        o = res.results[core]["out"].astype(np.float32)
        out[b] += o.transpose(1, 0, 2).reshape(D, T).T
    out += bo[None, None, :]
    return out


if __name__ == "__main__":
    nc = build_program()
    print("built ok")
